# revision 80
# baseline (speedup 1.0000x reference)
"""Causal multi-head attention on 8 Trainium2 NeuronCores (Bass/Tile).

Problem (hardcoded): x[2,2048,1024], W_qkv[1024,3072], b_qkv[3072],
W_proj[1024,1024], b_proj[1024]; 16 heads, head_dim 64, causal softmax.

Sharding: tensor-parallel over heads — core c owns heads (2c, 2c+1).
Each core computes qkv for its 2 heads (needs full x), the causal
attention for those heads, and a row-parallel partial of the output
projection. Host sums the 8 partials and adds the (precomputable) bias
terms.

Layout/dtype choices:
  - fp16 on the wire and in SBUF (PSUM accumulation stays fp32):
    halves DMA traffic and doubles DVE throughput vs f32/f32r, at
    identical PE matmul rate.
  - the qkv/v projections run in fp8e4m3 DoubleRow mode (2 contraction
    subtiles per instruction at 0.5 cycles/row, 4x the f32r rate). A
    host-side hi+lo fp8 decomposition of x and W (x ~= hi + lo, three
    cross terms, lo*lo dropped) keeps the relative error ~1e-3.
    Weights are pre-scaled x32 into fp8's normal range; the descale is
    fused into the bias add (tensor_scalar mult+add).
  - q,k are produced transposed (qT/kT [128=2*64, 4096]) straight out
    of the qkv matmul; v is produced directly in [token, feat] layout
    by swapping matmul operands (stationary = xT subtile), so no PE
    transposes are needed at all.
  - v blocks live interleaved as v_all[128, 32, 3, 64]: slot 0 = head0
    v, slot 1 = ones (softmax-denominator replicator), slot 2 = head1
    v, so both heads' PV stationaries are contiguous [2,64] slices
    (head 1 reads its numerator from PSUM partitions 64:128).
  - attention scores are computed as S^T = k @ q^T in [tk, tq] blocks
    of [128, 2, 512]; for diagonal superblocks only the unmasked
    column range [dlt*128, 512) is computed (matmul, exp AND PV all
    trimmed); the triangle sub-block is a 0/1 multiply on VectorE.
  - PV accumulation for trimmed diagonal blocks uses two matmuls (the
    triangle strip + the full remainder) with skip_group_check (region
    -granular start/stop is finer than the PSUM zero-region tracker).
  - emission is software-pipelined: qkv/v (A) and proj (C) work is cut
    into ~0.2-1.3us PE quanta pumped between attention j-steps so the
    in-order PE queue always has ready work while ScalarE runs exp;
    chunk x DMAs are issued 2 chunks ahead of consumption; junk
    warm-up matmuls burn the PE p-state ramp during the initial DMA
    fill; the trailing proj drain gets its own PSUM pool and
    split-engine PSUM drains once the attention pools retire.
"""

import numpy as np

import concourse.bass as bass
import concourse.tile as tile
from concourse import bacc, mybir
from concourse.bass_utils import run_bass_kernel_spmd

B, T, C = 2, 2048, 1024
H, D = 16, 64
TOK = B * T            # 4096
P = 128
NQ = 512               # q-chunk (moving free dim per head)
KB = 128               # k-block (PSUM partition dim)
KO = C // P            # 8 contraction subtiles
NCHUNK = TOK // NQ     # 8 token chunks
QC = T // NQ           # 4 q-chunks per batch
KBB = T // KB          # 16 k-blocks per batch
F32 = mybir.dt.float32
F16 = mybir.dt.float16
WSCALE = 32.0   # fp8 weight pre-scale (keeps hi/lo parts out of denormals)

_CACHE = {}
TRIM = True      # trim masked columns out of S/exp/PV on diagonal blocks
VDIRECT = True   # compute v via stationary-xT matmuls (strided PV weights)


def _build():
    nc = bacc.Bacc("TRN2", target_bir_lowering=False, debug=False, num_devices=8)
    marks = []
    _CACHE["marks"] = marks

    def mark(lbl):
        marks.append((nc.next_id(), lbl))

    F8 = mybir.dt.float8e4
    DR = mybir.MatmulPerfMode.DoubleRow
    xth_d = nc.dram_tensor("xth", [C, TOK], F8, kind="ExternalInput").ap()
    xtl_d = nc.dram_tensor("xtl", [C, TOK], F8, kind="ExternalInput").ap()
    wqkh_d = nc.dram_tensor("wqkh", [C, 256], F8, kind="ExternalInput").ap()
    wqkl_d = nc.dram_tensor("wqkl", [C, 256], F8, kind="ExternalInput").ap()
    bqk_d = nc.dram_tensor("bqk", [P, 2], F32, kind="ExternalInput").ap()
    wvh_d = nc.dram_tensor("wvh", [C, P], F8, kind="ExternalInput").ap()
    wvl_d = nc.dram_tensor("wvl", [C, P], F8, kind="ExternalInput").ap()
    wproj_d = nc.dram_tensor("wproj", [P, C], F16, kind="ExternalInput").ap()
    masks_d = nc.dram_tensor("masks", [P, P], F16, kind="ExternalInput").ap()
    y_d = nc.dram_tensor("y", [TOK, C], F16, kind="ExternalOutput").ap()

    with tile.TileContext(nc) as tc:
        with tc.tile_pool(name="res", bufs=1) as res, \
             tc.tile_pool(name="xt", bufs=4) as xtp, \
             tc.tile_pool(name="pt", bufs=8) as ptp, \
             tc.tile_pool(name="ystage", bufs=10) as ysp:
            # ---- resident tensors ----
            # qkv weights live as fp8 hi+lo pairs (host-decomposed, x32
            # scaled); the matmuls run in DoubleRow mode at 0.5 cycles/row.
            wqkh_sb = res.tile([P, KO, 256], F8, tag="wqkh")
            wqkl_sb = res.tile([P, KO, 256], F8, tag="wqkl")
            bqk_sb = res.tile([P, 2], F32, tag="bqk")
            nc.sync.dma_start(bqk_sb[:], bqk_d[:])
            nc.sync.dma_start(wqkh_sb[:],
                              wqkh_d.rearrange("(ko p) m -> p ko m", p=P))
            nc.sync.dma_start(wqkl_sb[:],
                              wqkl_d.rearrange("(ko p) m -> p ko m", p=P))
            wvh_sb = res.tile([P, KO, P], F8, tag="wvh")
            wvl_sb = res.tile([P, KO, P], F8, tag="wvl")
            wproj_sb = res.tile([P, C], F16, tag="wproj")
            masks_sb = res.tile([P, P], F16, tag="masks")

            qT_sb = res.tile([P, TOK], F16, tag="qT")
            kT_sb = res.tile([P, TOK], F16, tag="kT")
            # v blocks: [tk, blk, (v_h0 | ones | v_h1), d] — ones in the
            # middle so both heads' PV stationaries are contiguous slices;
            # head 1 reads its numerator from PSUM partitions 64:128.
            v_all = res.tile([P, 2 * KBB, 3, D], F16, tag="v_all")
            attns_sb = res.tile([P, TOK], F16, tag="attns")

            nc.gpsimd.memset(v_all[:, :, 1, :], 1.0)

            # ---- filler machinery: A(qkv+v) and C(proj) work is split into
            # small PE quanta pumped between attention j-steps, so the PE
            # (in-order queue) always has ready work while ScalarE runs exp.
            from collections import deque
            fill_q = deque()          # deque of (kind, closure)
            a_left_box = [0]

            def pump(k=1):
                n = 0
                while n < k and fill_q:
                    kind, f = fill_q.popleft()
                    if kind == "A":
                        a_left_box[0] -= 1
                    f()
                    n += 1

            # chunk xt DMAs are decoupled from the quantum pump and issued
            # 2 chunks ahead of consumption, so pumped filler matmuls never
            # head-of-line-block the PE queue waiting on their own DMA.
            chunk_st = [dict() for _ in range(NCHUNK)]
            dma_next = [0]

            def ensure_dma(upto):
                while dma_next[0] <= min(upto, NCHUNK - 1):
                    n = dma_next[0]
                    xth = xtp.tile([P, KO, NQ], F8, name="xth")
                    xtl = xtp.tile([P, KO, NQ], F8, name="xtl")
                    srch = xth_d.rearrange("(ko p) m -> p ko m", p=P)
                    srcl = xtl_d.rearrange("(ko p) m -> p ko m", p=P)
                    nc.sync.dma_start(xth[:], srch[:, :, n * NQ:(n + 1) * NQ])
                    nc.sync.dma_start(xtl[:], srcl[:, :, n * NQ:(n + 1) * NQ])
                    chunk_st[n]["xth"] = xth
                    chunk_st[n]["xtl"] = xtl
                    dma_next[0] += 1

            def make_A_quanta(n, psF):
                st = chunk_st[n]

                def qk_terms(mcols):
                    # (stationary, moving) hi/lo cross terms; hi*hi first so
                    # chunk 0 can start on the streaming hi tiles
                    return [
                        (wqkh_sb[:, :, mcols], lambda: st["xth"]),
                        (wqkh_sb[:, :, mcols], lambda: st["xtl"]),
                        (wqkl_sb[:, :, mcols], lambda: st["xth"]),
                    ]

                def emit_qk_mms(pq, mcols):
                    terms = qk_terms(mcols)
                    for t, (wsb, xf) in enumerate(terms):
                        xsb = xf()
                        for kp in range(KO // 2):
                            nc.tensor.matmul(
                                pq[:], wsb[:, 2 * kp:2 * kp + 2, :],
                                xsb[:, 2 * kp:2 * kp + 2, :],
                                start=(t == 0 and kp == 0),
                                stop=(t == 2 and kp == KO // 2 - 1),
                                perf_mode=DR)

                def q_qk01():
                    # chunk 0 gates the first attention chunk: interleave the
                    # q and k matmuls per k-pair so compute tracks the DMA
                    # stream, then run the two bias adds on both engines.
                    def f():
                        mark(f"A{n}.qk01")
                        ensure_dma(n + 2)
                        pq = psF.tile([P, NQ], F32, tag="f0", name="pq")
                        pk = psF.tile([P, NQ], F32, tag="f1", name="pk")
                        tq = qk_terms(slice(0, P))
                        tk = qk_terms(slice(P, 2 * P))
                        for t in range(3):
                            wq, xfq = tq[t]
                            wk, xfk = tk[t]
                            for kp in range(KO // 2):
                                nc.tensor.matmul(
                                    pq[:], wq[:, 2 * kp:2 * kp + 2, :],
                                    xfq()[:, 2 * kp:2 * kp + 2, :],
                                    start=(t == 0 and kp == 0),
                                    stop=(t == 2 and kp == KO // 2 - 1),
                                    perf_mode=DR)
                                nc.tensor.matmul(
                                    pk[:], wk[:, 2 * kp:2 * kp + 2, :],
                                    xfk()[:, 2 * kp:2 * kp + 2, :],
                                    start=(t == 0 and kp == 0),
                                    stop=(t == 2 and kp == KO // 2 - 1),
                                    perf_mode=DR)
                        nc.scalar.activation(
                            qT_sb[:, n * NQ:(n + 1) * NQ], pq[:],
                            mybir.ActivationFunctionType.Identity,
                            bias=bqk_sb[:, 0:1], scale=1.0 / WSCALE)
                        nc.vector.tensor_scalar(
                            kT_sb[:, n * NQ:(n + 1) * NQ], pk[:],
                            1.0 / WSCALE, bqk_sb[:, 1:2],
                            mybir.AluOpType.mult, mybir.AluOpType.add)
                    return f

                def q_qk(m):
                    def f():
                        mark(f"A{n}.qk{m}")
                        ensure_dma(n + 2)
                        pq = psF.tile([P, NQ], F32, tag=f"f{m}", name="pq")
                        emit_qk_mms(pq, slice(m * P, (m + 1) * P))
                        dst = qT_sb if m == 0 else kT_sb
                        nc.vector.tensor_scalar(
                            dst[:, n * NQ:(n + 1) * NQ], pq[:],
                            1.0 / WSCALE, bqk_sb[:, m:m + 1],
                            mybir.AluOpType.mult, mybir.AluOpType.add)
                    return f

                def q_v(m2):
                    # v for both heads of one 128-token block, in natural
                    # [token, feat] layout: stationary = xT subtile.
                    def f():
                        mark(f"A{n}.v{m2}")
                        pv = psF.tile([P, NQ], F32, tag=f"f{m2 % 2}", name="pv")
                        tc2 = m2 * P
                        terms = [
                            (st["xth"], wvh_sb), (st["xtl"], wvh_sb),
                            (st["xth"], wvl_sb),
                        ]
                        for t, (xsb, wsb) in enumerate(terms):
                            for kp in range(KO // 2):
                                nc.tensor.matmul(
                                    pv[:, 0:P],
                                    xsb[:, 2 * kp:2 * kp + 2, tc2:tc2 + P],
                                    wsb[:, 2 * kp:2 * kp + 2, :],
                                    start=(t == 0 and kp == 0),
                                    stop=(t == 2 and kp == KO // 2 - 1),
                                    perf_mode=DR)
                        nc.vector.tensor_scalar_mul(
                            v_all[:, n * 4 + m2, 0:3:2, :], pv[:, 0:P],
                            1.0 / WSCALE)
                    return f

                if n == 0:
                    return [("A", q_qk01()), ("A", lambda: None),
                            ("A", q_v(0)), ("A", q_v(1)), ("A", q_v(2)),
                            ("A", q_v(3))]
                return [("A", q_qk(0)), ("A", q_qk(1)),
                        ("A", q_v(0)), ("A", q_v(1)), ("A", q_v(2)),
                        ("A", q_v(3))]

            ys_box = {}
            ys_pre = set()   # m whose n2=0 half was staged but not yet DMA'd
            # C-quantum PSUM source: swapped to the wider drain pool (and
            # split-engine copies) once attention PSUM pools retire.
            c_mode = {"tile": None, "drain": False}

            def make_C_quantum(m, n2):
                # one proj matmul each; DVE and Act alternate the PSUM
                # drain copies so neither engine gates the tail
                def f():
                    mark(f"C.m{m}.{n2}")
                    if n2 == 0:
                        ys_box[m] = ysp.tile([P, 2, NQ], F16, name="ys")
                    ys = ys_box[m]
                    py = c_mode["tile"](n2)
                    nc.tensor.matmul(
                        py[:], attns_sb[:, m * P:(m + 1) * P],
                        wproj_sb[:, n2 * NQ:(n2 + 1) * NQ],
                        start=True, stop=True)
                    if c_mode["drain"]:
                        # tail: split the PSUM drain across both engines so
                        # the bank frees in ~half the time, and DMA each
                        # half-row out as soon as it is staged
                        nc.vector.tensor_copy(
                            ys[:, n2, 0:NQ // 2], py[:, 0:NQ // 2])
                        nc.scalar.copy(
                            ys[:, n2, NQ // 2:NQ], py[:, NQ // 2:NQ])
                        if n2 == 1:
                            nc.sync.dma_start(
                                y_d[m * P:(m + 1) * P, :], ys_box.pop(m)[:])
                    else:
                        if n2 == 0:
                            nc.vector.tensor_copy(ys[:, n2, :], py[:])
                            ys_pre.add(m)
                        else:
                            nc.scalar.copy(ys[:, n2, :], py[:])
                            ys_pre.discard(m)
                            nc.sync.dma_start(
                                y_d[m * P:(m + 1) * P, :], ys_box.pop(m)[:])
                return f

            # ---- stage B chunk: attention for batch b, q-chunk i ----
            js_left_box = [80]  # total j-steps over all B chunks

            def emit_B(b, i):
                nq0 = b * T + i * NQ
                jmax = 4 * i + 4
                psS, psO = psS_g, psO_g
                po = [psO.tile([P, NQ], F32, tag=f"o{h}", name=f"po{h}")
                      for h in range(2)]
                s_tiles = {}

                def emit_s(j):
                    s = psS.tile([P, 2, NQ], F32, tag="s", name="s")
                    dlt = max(0, j - 4 * i) if TRIM else 0
                    for h in range(2):
                        nc.tensor.matmul(
                            s[:, h, dlt * KB:NQ],
                            kT_sb[h * D:(h + 1) * D,
                                  b * T + j * KB: b * T + (j + 1) * KB],
                            qT_sb[h * D:(h + 1) * D,
                                  nq0 + dlt * KB:nq0 + NQ],
                            start=True, stop=True)
                    s_tiles[j] = s

                emit_s(0)
                budget0 = len(fill_q) * jmax // js_left_box[0]
                js_left_box[0] -= jmax
                taken = 0
                for j in range(jmax):
                    mark(f"B{b}.{i}.j{j}")
                    if j + 1 < jmax:
                        emit_s(j + 1)
                    pt = ptp.tile([P, 2, NQ], F16, name="pt")
                    s = s_tiles.pop(j)
                    blk = b * KBB + j
                    if j >= 4 * i:
                        # diagonal superblock: columns < dlt*KB are fully
                        # masked and skipped end-to-end; the triangle is one
                        # KB-wide sub-block masked via 0/1 multiply. One exp
                        # and one mask-mul instruction cover both heads.
                        rd = j - 4 * i
                        dlt = rd if TRIM else 0
                        nc.scalar.activation(
                            pt[:, :, dlt * KB:NQ], s[:, :, dlt * KB:NQ],
                            mybir.ActivationFunctionType.Exp)
                        mb = masks_sb[:].unsqueeze(1).broadcast_to([P, 2, KB])
                        nc.vector.tensor_mul(
                            pt[:, :, rd * KB:(rd + 1) * KB],
                            pt[:, :, rd * KB:(rd + 1) * KB], mb)
                        if not TRIM and rd > 0:
                            nc.vector.tensor_scalar_mul(
                                pt[:, :, 0:rd * KB],
                                pt[:, :, 0:rd * KB], 0.0)
                    else:
                        nc.scalar.activation(
                            pt[:], s[:],
                            mybir.ActivationFunctionType.Exp)
                    want = max(budget0 * (j + 1) // jmax, j + 1)
                    if want > taken:
                        pump(want - taken)
                        taken = want
                    for h in range(2):
                        vst = (v_all[:, blk, 0:2, :] if h == 0
                               else v_all[:, blk, 1:3, :])
                        # head 0: PSUM partitions 0:64 = numerator, 64:128 =
                        # denominator; head 1: swapped (ones slot first).
                        nu0, de0 = (0, D) if h == 0 else (D, 0)
                        if TRIM and j >= 4 * i:
                            dlt = j - 4 * i
                            # remainder first (start covers the whole bank's
                            # zero region at j==0), then the triangle strip
                            # with stop on the strip's last contribution.
                            if dlt < 3:
                                nc.tensor.matmul(
                                    po[h][:, (dlt + 1) * KB:NQ], vst,
                                    pt[:, h, (dlt + 1) * KB:NQ],
                                    start=(j == 0), stop=(j == jmax - 1),
                                    skip_group_check=True)
                            nc.tensor.matmul(
                                po[h][:, dlt * KB:(dlt + 1) * KB], vst,
                                pt[:, h, dlt * KB:(dlt + 1) * KB],
                                start=False, stop=True,
                                skip_group_check=True)
                        elif not TRIM:
                            nc.tensor.matmul(
                                po[h][:], vst, pt[:, h, :],
                                start=(j == 0), stop=(j == jmax - 1))
                        else:
                            nc.tensor.matmul(
                                po[h][:], vst, pt[:, h, :],
                                start=(j == 0), stop=False,
                                skip_group_check=True)
                        if j == jmax - 1:
                            # normalize this head immediately: its recip
                            # runs on DVE while PE starts the other head
                            rc = ptp.tile([D, NQ], F32, tag="rc", name="rc",
                                          bufs=2)
                            nc.vector.reciprocal(
                                rc[:], po[h][de0:de0 + D, :])
                            nc.vector.tensor_mul(
                                attns_sb[h * D:(h + 1) * D, nq0:nq0 + NQ],
                                po[h][nu0:nu0 + D, :], rc[:])

            # ---- interleaved emission ----
            with tc.tile_pool(name="psF", bufs=1, space="PSUM") as psF:
                c_mode["tile"] = lambda n2: psF.tile(
                    [P, NQ], F32, tag=f"f{n2}", name="py")
                with tc.tile_pool(name="psS", bufs=2, space="PSUM") as psS_g, \
                     tc.tile_pool(name="psO", bufs=1, space="PSUM") as psO_g:
                    for n in range(NCHUNK):
                        for kq in make_A_quanta(n, psF):
                            fill_q.append(kq)
                            a_left_box[0] += 1

                    # prologue: bqk+wqk already queued first; chunk-0 xt next
                    # (8 HWDGE-paced k-tiles), then wv behind them
                    ensure_dma(0)
                    nc.sync.dma_start(
                        wvh_sb[:], wvh_d.rearrange("(ko p) m -> p ko m", p=P))
                    nc.sync.dma_start(
                        wvl_sb[:], wvl_d.rearrange("(ko p) m -> p ko m", p=P))
                    # warm-up: junk matmuls on a memset tile burn the PE
                    # p-state ramp while the first x chunk is still in
                    # flight; a tiny Identity activation hoists the 1.3us
                    # activation-table load off the first bias-add.
                    jnk = res.tile([P, NQ], F16, tag="jnk")
                    nc.vector.memset(jnk[:], 0.5)
                    nc.scalar.activation(
                        jnk[:, 0:1], jnk[:, 0:1],
                        mybir.ActivationFunctionType.Identity)
                    pj = psS_g.tile([P, 2, NQ], F32, tag="s", name="pj")
                    for r in range(6):
                        nc.tensor.matmul(
                            pj[:, r % 2, :], jnk[:, 0:P], jnk[:],
                            start=True, stop=True)
                    pump(7)
                    nc.sync.dma_start(masks_sb[:], masks_d[:])

                    # chunk order: small chunk last -> short tail; module
                    # flag allows schedule experiments
                    corder = _CACHE.get(
                        "corder",
                        [(0, 0), (0, 1), (0, 2), (0, 3),
                         (1, 1), (1, 2), (1, 3), (1, 0)])
                    if True:
                        for b, i in corder:
                            # A chunks needed by this B chunk first
                            need = 6 * (NCHUNK - (b * QC + i + 1))
                            while a_left_box[0] > need:
                                pump(1)
                            if _CACHE.get("dbg_sched"):
                                print(f"emit_B({b},{i}) jmax={4*i+4} "
                                      f"fill_q={len(fill_q)} "
                                      f"a_left={a_left_box[0]}")
                            emit_B(b, i)
                            if b == 0 and i == 0:
                                nc.sync.dma_start(wproj_sb[:], wproj_d[:])
                            # each chunk's proj quanta only need that chunk's
                            # attns columns: queue them as soon as the chunk
                            # is normalized, so A quanta survive as fillers
                            # for the late (filler-starved) phase
                            for m in range(b * 16 + 4 * i, b * 16 + 4 * i + 4):
                                for n2 in range(2):
                                    fill_q.append(
                                        ("C", make_C_quantum(m, n2)))
                # trailing drain: attention PSUM pools are closed, so give
                # the leftover proj quanta more banks + split-engine drains
                with tc.tile_pool(name="psD", bufs=1, space="PSUM") as psD:
                    nd = [0]

                    def drain_tile(n2):
                        t = psD.tile([P, NQ], F32, tag=f"d{nd[0] % 4}",
                                     name="pyd")
                        nd[0] += 1
                        return t
                    c_mode["tile"] = drain_tile
                    c_mode["drain"] = True
                    if _CACHE.get("dbg_sched"):
                        print(f"drain: {len(fill_q)} quanta left")
                    while fill_q:
                        kind, f = fill_q.popleft()
                        f()

    nc.compile()
    return nc


def _host_prep(x, W_qkv, b_qkv, W_proj, b_proj):
    x = np.asarray(x, dtype=np.float32)
    W_qkv = np.asarray(W_qkv, dtype=np.float32)
    b_qkv = np.asarray(b_qkv, dtype=np.float32)
    W_proj = np.asarray(W_proj, dtype=np.float32)
    b_proj = np.asarray(b_proj, dtype=np.float32)

    import ml_dtypes
    F8NP = ml_dtypes.float8_e4m3

    def hilo(a):
        # fp8 hi + lo decomposition: a ~= hi + lo to ~0.1% relative
        hi = a.astype(F8NP)
        lo = (a - hi.astype(np.float32)).astype(F8NP)
        return np.ascontiguousarray(hi), np.ascontiguousarray(lo)

    xT = np.ascontiguousarray(x.reshape(TOK, C).T)
    xth, xtl = hilo(xT)
    scale = np.float32(1.0 / np.sqrt(D))

    masks = np.ascontiguousarray(
        np.triu(np.ones((P, P), dtype=np.float16)))  # [tk, tq]: tq >= tk

    in_maps = []
    for c in range(8):
        s0, s1 = c * P, (c + 1) * P
        wq = W_qkv[:, s0:s1] * scale
        wk = W_qkv[:, C + s0:C + s1]
        wv = W_qkv[:, 2 * C + s0:2 * C + s1]
        bq = b_qkv[s0:s1] * scale
        bk = b_qkv[C + s0:C + s1]
        wqkh, wqkl = hilo(
            np.concatenate([wq, wk], axis=1) * np.float32(WSCALE))
        wvh, wvl = hilo(wv * np.float32(WSCALE))
        in_maps.append({
            "xth": xth,
            "xtl": xtl,
            "wqkh": wqkh,
            "wqkl": wqkl,
            "bqk": np.ascontiguousarray(np.stack([bq, bk], axis=1)),
            "wvh": wvh,
            "wvl": wvl,
            "wproj": np.ascontiguousarray(W_proj[s0:s1, :].astype(np.float16)),
            "masks": masks,
        })
    # constant bias terms folded on host:
    #   out_proj bias + (v-bias row) @ W_proj  (v bias passes through softmax)
    ybias = b_qkv[2 * C:3 * C] @ W_proj + b_proj  # [1024]
    return in_maps, ybias


def kernel(x, W_qkv, b_qkv, W_proj, b_proj):
    if "nc" not in _CACHE:
        _CACHE["nc"] = _build()
    nc = _CACHE["nc"]
    in_maps, ybias = _host_prep(x, W_qkv, b_qkv, W_proj, b_proj)
    try:
        res = run_bass_kernel_spmd(nc, in_maps, core_ids=list(range(8)))
    except Exception:
        # transient device errors (NRT_EXEC_UNIT_UNRECOVERABLE) heal on retry
        res = run_bass_kernel_spmd(nc, in_maps, core_ids=list(range(8)))
    y = np.zeros((TOK, C), dtype=np.float32)
    for c in range(8):
        y += res.results[c]["y"].astype(np.float32)
    y += ybias[None, :].astype(np.float32)
    return y.reshape(B, T, C)
